# revision 28
# baseline (speedup 1.0000x reference)
"""nn_CrossAttention_tau — Trainium2 Bass kernel, 8-core data/head parallel.

Sharding: B=4 batches x 12 heads -> 8 cores, each core owns 1 batch x 6 heads
(3 head-pairs). Full inputs in, full output out; host does layout
(transposes/slicing) + final gather only.

Per-core device program (identical NEFF, per-core input data):
  phase 0: tau = softplus(tau_param)+1e-6 on device; scale = D^-0.5/tau
  phase 1 (prelude): V = y @ Wv^T (natural [m,d]), K^T, Q^T via PE
  phase 2 (pairs):   per head-pair, per q-half(1024), per m-chunk(128):
                       S^T[m,q] = K^T.T @ Q^T (row-tiled K=64 pair)
                       E = exp(S^T * scale)  (ACT, PSUM->SBUF)
                       O += V.T @ E (col-tiled pair) ; rowsums via ones-lhsT
                     normalize: O * (1/rowsum) -> bf16 O_norm
  phase 3 (tail):    out^T = Wp_slice^T.T @ O_norm (bf16), DMA out
Host: out[b] = core(2b).T + core(2b+1).T + bproj
"""

import os

import numpy as np

import concourse.bacc as bacc
import concourse.mybir as mybir
import concourse.tile as tile
from concourse.bass_utils import run_bass_kernel_spmd

B, N, C, H, D = 4, 2048, 768, 12, 64
HPC = H // 2  # heads per core = 6
PAIRS = 3  # head pairs per core
F32 = mybir.dt.float32
BF16 = mybir.dt.bfloat16
NB = 4  # 512-wide q/n blocks
MC = N // 128  # 16 m-chunks
CC = C // 128  # 6 contraction chunks
WQKV_W = 3 * HPC * D  # 1152


DEBUG_DUMP = bool(int(os.environ.get("KERNEL_DEBUG", "0")))


def _build():
    nc = bacc.Bacc()
    xT = nc.dram_tensor("xT", [C, N], F32, kind="ExternalInput")
    yT = nc.dram_tensor("yT", [C, N], F32, kind="ExternalInput")
    wqkvT = nc.dram_tensor("wqkvT", [C, WQKV_W], F32, kind="ExternalInput")
    wp = nc.dram_tensor("wp", [128, PAIRS * C], BF16, kind="ExternalInput")
    tau_in = nc.dram_tensor("tau_in", [1, 1], F32, kind="ExternalInput")
    outT = nc.dram_tensor("outT", [C, N], F32, kind="ExternalOutput")
    if DEBUG_DUMP:
        dbg_qT = nc.dram_tensor("dbg_qT", [128, N], F32, kind="ExternalOutput")
        dbg_kT = nc.dram_tensor("dbg_kT", [128, N], F32, kind="ExternalOutput")
        dbg_v = nc.dram_tensor("dbg_v", [128, N], F32, kind="ExternalOutput")
        dbg_e = nc.dram_tensor("dbg_e", [128, 1024], F32, kind="ExternalOutput")
        dbg_rs = nc.dram_tensor("dbg_rs", [128, 512], F32, kind="ExternalOutput")
        dbg_on = nc.dram_tensor("dbg_on", [128, N], BF16, kind="ExternalOutput")
        dbg_scale = nc.dram_tensor("dbg_scale", [128, 1], F32, kind="ExternalOutput")
        dbg_rr = nc.dram_tensor("dbg_rr", [1, 512], F32, kind="ExternalOutput")
        dbg_bc = nc.dram_tensor("dbg_bc", [128, 512], F32, kind="ExternalOutput")
        dbg_ou = nc.dram_tensor("dbg_ou", [128, 512], F32, kind="ExternalOutput")
        dbg_bc2 = nc.dram_tensor("dbg_bc2", [128, 512], F32, kind="ExternalOutput")

    Exp = mybir.ActivationFunctionType.Exp
    Ln = mybir.ActivationFunctionType.Ln

    with tile.TileContext(nc) as tc:
        import contextlib

        with contextlib.ExitStack() as ctx:
            consts = ctx.enter_context(tc.tile_pool(name="consts", bufs=1))
            wpool = ctx.enter_context(tc.tile_pool(name="wpool", bufs=1))
            xy = ctx.enter_context(tc.tile_pool(name="xy", bufs=6))
            qkv = ctx.enter_context(tc.tile_pool(name="qkv", bufs=1))
            epool = ctx.enter_context(tc.tile_pool(name="epool", bufs=2))
            onorm = ctx.enter_context(tc.tile_pool(name="onorm", bufs=1))
            npool = ctx.enter_context(tc.tile_pool(name="npool", bufs=1))
            stage = ctx.enter_context(tc.tile_pool(name="stage", bufs=3))

            # ---- phase 0: constants ------------------------------------
            ones = consts.tile([128, 1], F32, tag="ones")
            nc.vector.memset(ones, 1.0)
            ones_row = consts.tile([1, 128], F32, tag="ones_row")
            nc.vector.memset(ones_row, 1.0)
            ones_full = consts.tile([128, 128], F32, tag="ones_full")
            nc.vector.memset(ones_full, 1.0)
            t_tau = consts.tile([1, 1], F32, tag="t_tau")
            nc.sync.dma_start(t_tau[:], tau_in[:])
            t_e = consts.tile([1, 1], F32, tag="t_e")
            nc.scalar.activation(t_e[:], t_tau[:], Exp)
            t_sp = consts.tile([1, 1], F32, tag="t_sp")
            nc.scalar.activation(t_sp[:], t_e[:], Ln, bias=1.0)
            t_sp2 = consts.tile([1, 1], F32, tag="t_sp2")
            nc.vector.tensor_scalar_add(t_sp2[:], t_sp[:], 1e-6)
            t_inv = consts.tile([1, 1], F32, tag="t_inv")
            nc.vector.reciprocal(t_inv[:], t_sp2[:])
            t_s1 = consts.tile([1, 1], F32, tag="t_s1")
            nc.vector.tensor_scalar_mul(t_s1[:], t_inv[:], float(D**-0.5))
            scale = consts.tile([128, 1], F32, tag="scale")
            with tc.tile_pool(name="ps_c", bufs=1, space="PSUM") as ps_c:
                sc_ps = ps_c.tile([128, 1], F32, tag="sc_ps")
                nc.tensor.matmul(sc_ps[:], ones_row[:], t_s1[:])
                nc.vector.tensor_copy(scale[:], sc_ps[:])

            # ---- weights ----------------------------------------------
            w_all = wpool.tile([128, CC, WQKV_W], F32, tag="w_all")
            for c in range(CC):
                nc.sync.dma_start(
                    w_all[:, c, :], wqkvT[c * 128 : (c + 1) * 128, :]
                )
            wp_sb = wpool.tile([128, PAIRS * C], BF16, tag="wp_sb")
            nc.sync.dma_start(wp_sb[:], wp[:])

            def wq_sl(c, p):
                return w_all[:, c, p * 128 : (p + 1) * 128]

            def wk_sl(c, p):
                off = HPC * D
                return w_all[:, c, off + p * 128 : off + (p + 1) * 128]

            def wv_sl(c):
                off = 2 * HPC * D
                return w_all[:, c, off : off + HPC * D]

            # resident Q^T/K^T/V tiles
            qT = [qkv.tile([128, N], F32, tag=f"qT{p}", name=f"qT{p}") for p in range(PAIRS)]
            kT = [qkv.tile([128, N], F32, tag=f"kT{p}", name=f"kT{p}") for p in range(PAIRS)]
            v_sb = [qkv.tile([128, N], F32, tag=f"v{p}", name=f"v{p}") for p in range(PAIRS)]
            o_nrm = [onorm.tile([128, N], BF16, tag=f"on{p}", name=f"on{p}") for p in range(PAIRS)]

            # ---- phase 1: prelude (V, K^T, Q^T) ------------------------
            with (
                tc.tile_pool(name="ps_pre", bufs=4, space="PSUM") as ps_pre,
                tc.tile_pool(name="ps_v", bufs=2, space="PSUM") as ps_v,
            ):
                yts = [xy.tile([128, N], F32, tag="xy", name="xy") for _ in range(CC)]
                for c in range(CC):
                    nc.sync.dma_start(yts[c][:], yT[c * 128 : (c + 1) * 128, :])

                # V natural: [m, d] accumulated over c
                for mc in range(MC):
                    pv = ps_v.tile([128, HPC * D], F32, tag="pv")
                    for c in range(CC):
                        nc.tensor.matmul(
                            pv[:],
                            yts[c][:, mc * 128 : (mc + 1) * 128],
                            wv_sl(c),
                            start=(c == 0),
                            stop=(c == CC - 1),
                        )
                    for p in range(PAIRS):
                        nc.vector.tensor_copy(
                            v_sb[p][:, mc * 128 : (mc + 1) * 128],
                            pv[:, p * 128 : (p + 1) * 128],
                        )

                # K^T
                for p in range(PAIRS):
                    pk = [ps_pre.tile([128, 512], F32, tag="pre", name="pre") for _ in range(NB)]
                    for c in range(CC):
                        for nb in range(NB):
                            nc.tensor.matmul(
                                pk[nb][:],
                                wk_sl(c, p),
                                yts[c][:, nb * 512 : (nb + 1) * 512],
                                start=(c == 0),
                                stop=(c == CC - 1),
                            )
                    for nb in range(NB):
                        nc.vector.tensor_copy(
                            kT[p][:, nb * 512 : (nb + 1) * 512], pk[nb][:]
                        )

                # Q^T (xT replaces yT in the xy pool)
                xts = [xy.tile([128, N], F32, tag="xy", name="xy") for _ in range(CC)]
                for c in range(CC):
                    nc.sync.dma_start(xts[c][:], xT[c * 128 : (c + 1) * 128, :])
                for p in range(PAIRS):
                    pq = [ps_pre.tile([128, 512], F32, tag="pre", name="pre") for _ in range(NB)]
                    for c in range(CC):
                        for nb in range(NB):
                            nc.tensor.matmul(
                                pq[nb][:],
                                wq_sl(c, p),
                                xts[c][:, nb * 512 : (nb + 1) * 512],
                                start=(c == 0),
                                stop=(c == CC - 1),
                            )
                    for nb in range(NB):
                        nc.vector.tensor_copy(
                            qT[p][:, nb * 512 : (nb + 1) * 512], pq[nb][:]
                        )

            if DEBUG_DUMP:
                nc.sync.dma_start(dbg_qT[:], qT[0][:])
                nc.sync.dma_start(dbg_kT[:], kT[0][:])
                nc.sync.dma_start(dbg_v[:], v_sb[0][:])
                nc.sync.dma_start(dbg_scale[:], scale[:])

            # ---- phase 2: attention pairs ------------------------------
            with (
                tc.tile_pool(name="ps_acc", bufs=1, space="PSUM") as ps_acc,
                tc.tile_pool(name="ps_s", bufs=1, space="PSUM") as ps_s,
                tc.tile_pool(name="ps_bc", bufs=1, space="PSUM") as ps_bc,
            ):
                for p in range(PAIRS):
                    for hf in range(2):  # q-halves of 1024
                        q0 = hf * 1024
                        o_qb = [
                            ps_acc.tile([128, 512], F32, tag=f"oq{qb}", name=f"oq{qb}")
                            for qb in range(2)
                        ]
                        rs = ps_acc.tile([128, 512], F32, tag="rs")
                        for mc in range(MC):
                            m0 = mc * 128
                            s_a = ps_s.tile([128, 1024], F32, tag="sa")
                            s_b = ps_s.tile([128, 1024], F32, tag="sb")
                            for qb in range(2):
                                nc.tensor.matmul(
                                    s_a[:, qb * 512 : (qb + 1) * 512],
                                    kT[p][0:64, m0 : m0 + 128],
                                    qT[p][0:64, q0 + qb * 512 : q0 + (qb + 1) * 512],
                                    tile_position=(0, 0),
                                )
                                nc.tensor.matmul(
                                    s_b[:, qb * 512 : (qb + 1) * 512],
                                    kT[p][64:128, m0 : m0 + 128],
                                    qT[p][64:128, q0 + qb * 512 : q0 + (qb + 1) * 512],
                                    tile_position=(64, 0),
                                )
                            e_a = epool.tile([128, 1024], F32, tag="ea")
                            e_b = epool.tile([128, 1024], F32, tag="eb")
                            nc.scalar.activation(
                                e_a[:], s_a[:], Exp, scale=scale[:]
                            )
                            nc.scalar.activation(
                                e_b[:], s_b[:], Exp, scale=scale[:]
                            )
                            if DEBUG_DUMP and p == 0 and hf == 0 and mc == 0:
                                nc.sync.dma_start(dbg_e[:], e_a[:])
                            st = dict(
                                start=(mc == 0),
                                stop=(mc == MC - 1),
                                skip_group_check=True,
                            )
                            for qb in range(2):
                                nc.tensor.matmul(
                                    o_qb[qb][0:64, :],
                                    v_sb[p][:, m0 : m0 + 64],
                                    e_a[:, qb * 512 : (qb + 1) * 512],
                                    tile_position=(0, 0),
                                    **st,
                                )
                            for qb in range(2):
                                nc.tensor.matmul(
                                    o_qb[qb][64:128, :],
                                    v_sb[p][:, m0 + 64 : m0 + 128],
                                    e_b[:, qb * 512 : (qb + 1) * 512],
                                    tile_position=(0, 64),
                                    **st,
                                )
                            # rowsums: rows 0/32 = qb0 (A/B), 64/96 = qb1
                            for qb in range(2):
                                for hd, e_t in ((0, e_a), (1, e_b)):
                                    row = qb * 64 + hd * 32
                                    nc.tensor.matmul(
                                        rs[row : row + 1, :],
                                        ones[:, 0:1],
                                        e_t[:, qb * 512 : (qb + 1) * 512],
                                        tile_position=(0, row),
                                        **st,
                                    )
                        if DEBUG_DUMP and p == 0 and hf == 0:
                            rs_st = stage.tile([128, 512], F32, tag="rs_st")
                            nc.vector.tensor_copy(rs_st[:], rs[:])
                            nc.sync.dma_start(dbg_rs[:], rs_st[:])
                            ou_st = stage.tile([128, 512], F32, tag="ou_st")
                            nc.vector.tensor_copy(ou_st[:], o_qb[0][:])
                            nc.sync.dma_start(dbg_ou[:], ou_st[:])
                        # normalize both q-blocks of this half
                        rr_all = npool.tile(
                            [128, 512], F32, tag="rr_all", name="rr_all"
                        )
                        nc.vector.reciprocal(
                            rr_all[0:97, :], rs[0:97, :]
                        )
                        for qb in range(2):
                            sl = slice(q0 + qb * 512, q0 + (qb + 1) * 512)
                            for hd in range(2):
                                row = qb * 64 + hd * 32
                                bc_ps = ps_bc.tile(
                                    [128, 512], F32, tag="bc_ps", name="bc_ps"
                                )
                                nc.tensor.matmul(
                                    bc_ps[:],
                                    ones_full[row : row + 1, 0:128],
                                    rr_all[row : row + 1, :],
                                    tile_position=(96, 0) if row == 96 else None,
                                )
                                bc = npool.tile(
                                    [128, 512], F32, tag=f"bc{hd}", name=f"bc{hd}"
                                )
                                nc.vector.tensor_copy(bc[:], bc_ps[:])
                                if (
                                    DEBUG_DUMP
                                    and p == 0
                                    and hf == 0
                                    and qb == 0
                                    and hd == 0
                                ):
                                    nc.sync.dma_start(dbg_rr[:], rr_all[0:1, :])
                                    nc.sync.dma_start(dbg_bc[:], bc[:])
                                if (
                                    DEBUG_DUMP
                                    and p == 0
                                    and hf == 0
                                    and qb == 1
                                    and hd == 0
                                ):
                                    nc.sync.dma_start(dbg_bc2[:], bc[:])
                                hs = slice(hd * 64, (hd + 1) * 64)
                                nc.vector.tensor_mul(
                                    o_nrm[p][hs, sl], o_qb[qb][hs, :], bc[hs, :]
                                )

            if DEBUG_DUMP:
                nc.sync.dma_start(dbg_on[:], o_nrm[0][:])

            # ---- phase 3: output projection ----------------------------
            with tc.tile_pool(name="ps_out", bufs=4, space="PSUM") as ps_out:
                for ic in range(CC):
                    for nb in range(NB):
                        po = ps_out.tile([128, 512], F32, tag="po")
                        for p in range(PAIRS):
                            nc.tensor.matmul(
                                po[:],
                                wp_sb[:, p * C + ic * 128 : p * C + (ic + 1) * 128],
                                o_nrm[p][:, nb * 512 : (nb + 1) * 512],
                                start=(p == 0),
                                stop=(p == PAIRS - 1),
                            )
                        so = stage.tile([128, 512], F32, tag="so")
                        nc.scalar.copy(so[:], po[:])
                        nc.sync.dma_start(
                            outT[
                                ic * 128 : (ic + 1) * 128,
                                nb * 512 : (nb + 1) * 512,
                            ],
                            so[:],
                        )
    nc.compile()
    return nc


_NC = None


def _get_nc():
    global _NC
    if _NC is None:
        _NC = _build()
    return _NC


def kernel(x, y, Wq, Wkv, tau_param, Wproj, bproj):
    x = np.asarray(x, np.float32)
    y = np.asarray(y, np.float32)
    Wq = np.asarray(Wq, np.float32)
    Wkv = np.asarray(Wkv, np.float32)
    Wproj = np.asarray(Wproj, np.float32)
    bproj = np.asarray(bproj, np.float32)
    tau_np = np.asarray(tau_param, np.float32).reshape(1, 1)

    import ml_dtypes

    in_maps = []
    for c in range(8):
        b = c // 2
        h0 = (c % 2) * HPC
        rows = slice(h0 * D, h0 * D + HPC * D)
        wq_s = Wq[rows, :].T  # [C, 384]
        wk_s = Wkv[rows, :].T
        wv_s = Wkv[C + h0 * D : C + h0 * D + HPC * D, :].T
        wqkvT = np.ascontiguousarray(
            np.concatenate([wq_s, wk_s, wv_s], axis=1), np.float32
        )
        wpT = Wproj[:, h0 * D : h0 * D + HPC * D].T  # [384, C]
        wp_packed = np.empty((128, PAIRS * C), ml_dtypes.bfloat16)
        for p in range(PAIRS):
            wp_packed[:, p * C : (p + 1) * C] = wpT[
                p * 128 : (p + 1) * 128, :
            ].astype(ml_dtypes.bfloat16)
        in_maps.append(
            {
                "xT": np.ascontiguousarray(x[b].T),
                "yT": np.ascontiguousarray(y[b].T),
                "wqkvT": wqkvT,
                "wp": wp_packed,
                "tau_in": tau_np,
            }
        )

    nc = _get_nc()
    trace = bool(int(os.environ.get("KERNEL_PROFILE", "0")))
    if trace:
        _install_ntff_shim()
    res = run_bass_kernel_spmd(nc, in_maps, list(range(8)), trace=trace)
    kernel.last_results = res.results
    if trace and res.exec_time_ns is not None:
        print(f"HW exec time: {res.exec_time_ns} ns")
        kernel.last_exec_time_ns = res.exec_time_ns
        kernel.last_trace = res.instructions_and_trace

    out = np.empty((B, N, C), np.float32)
    for b in range(B):
        acc = res.results[2 * b]["outT"].T + res.results[2 * b + 1]["outT"].T
        out[b] = acc + bproj[None, :]
    return out


def _install_ntff_shim():
    import sys
    import types

    try:
        from antenv import axon_hooks  # noqa: F401

        return
    except ImportError:
        pass
    from trn_agent_boot.trn_boot import _ntff_profile_via_ctypes

    hook = _ntff_profile_via_ctypes("/opt/axon/libaxon_pjrt.so")
    mod = types.ModuleType("antenv.axon_hooks")
    mod.get_axon_ntff_profile_hook = lambda: hook
    mod.set_axon_ntff_profile_hook = lambda h: None
    sys.modules["antenv.axon_hooks"] = mod
    import concourse.bass_utils as bu

    bu.upload_artifacts = lambda tmpdir: "local://" + str(tmpdir)


# revision 30
# speedup vs baseline: 2.0739x; 2.0739x over previous
"""nn_CrossAttention_tau — Trainium2 Bass kernel, 8-core data/head parallel.

Sharding: B=4 batches x 12 heads -> 8 cores, each core owns 1 batch x 6 heads
(3 head-pairs). Full inputs in, full output out; host does layout
(transposes/slicing) + final gather only.

Per-core device program (identical NEFF, per-core input data):
  phase 0: tau = softplus(tau_param)+1e-6 on device; scale = D^-0.5/tau
  phase 1 (prelude): V = y @ Wv^T (natural [m,d]), K^T, Q^T via PE
  phase 2 (pairs):   per head-pair, per q-half(1024), per m-chunk(128):
                       S^T[m,q] = K^T.T @ Q^T (row-tiled K=64 pair)
                       E = exp(S^T * scale)  (ACT, PSUM->SBUF)
                       O += V.T @ E (col-tiled pair) ; rowsums via ones-lhsT
                     normalize: O * (1/rowsum) -> bf16 O_norm
  phase 3 (tail):    out^T = Wp_slice^T.T @ O_norm (bf16), DMA out
Host: out[b] = core(2b).T + core(2b+1).T + bproj
"""

import os

import numpy as np

import concourse.bacc as bacc
import concourse.mybir as mybir
import concourse.tile as tile
from concourse.bass_utils import run_bass_kernel_spmd

B, N, C, H, D = 4, 2048, 768, 12, 64
HPC = H // 2  # heads per core = 6
PAIRS = 3  # head pairs per core
F32 = mybir.dt.float32
BF16 = mybir.dt.bfloat16
NB = 4  # 512-wide q/n blocks
MC = N // 128  # 16 m-chunks
CC = C // 128  # 6 contraction chunks
WQKV_W = 3 * HPC * D  # 1152


DEBUG_DUMP = bool(int(os.environ.get("KERNEL_DEBUG", "0")))


def _build():
    nc = bacc.Bacc()
    xT = nc.dram_tensor("xT", [C, N], BF16, kind="ExternalInput")
    yT = nc.dram_tensor("yT", [C, N], BF16, kind="ExternalInput")
    wqkvT = nc.dram_tensor("wqkvT", [C, WQKV_W], BF16, kind="ExternalInput")
    wp = nc.dram_tensor("wp", [128, PAIRS * C], BF16, kind="ExternalInput")
    tau_in = nc.dram_tensor("tau_in", [1, 1], F32, kind="ExternalInput")
    outT = nc.dram_tensor("outT", [C, N], F32, kind="ExternalOutput")
    if DEBUG_DUMP:
        dbg_qT = nc.dram_tensor("dbg_qT", [128, N], BF16, kind="ExternalOutput")
        dbg_kT = nc.dram_tensor("dbg_kT", [128, N], BF16, kind="ExternalOutput")
        dbg_v = nc.dram_tensor("dbg_v", [128, N], BF16, kind="ExternalOutput")
        dbg_e = nc.dram_tensor("dbg_e", [128, 1024], BF16, kind="ExternalOutput")
        dbg_rs = nc.dram_tensor("dbg_rs", [128, 512], F32, kind="ExternalOutput")
        dbg_on = nc.dram_tensor("dbg_on", [128, N], BF16, kind="ExternalOutput")
        dbg_scale = nc.dram_tensor("dbg_scale", [128, 1], F32, kind="ExternalOutput")
        dbg_rr = nc.dram_tensor("dbg_rr", [1, 512], F32, kind="ExternalOutput")
        dbg_bc = nc.dram_tensor("dbg_bc", [128, 512], F32, kind="ExternalOutput")
        dbg_ou = nc.dram_tensor("dbg_ou", [128, 512], F32, kind="ExternalOutput")
        dbg_bc2 = nc.dram_tensor("dbg_bc2", [128, 512], F32, kind="ExternalOutput")

    Exp = mybir.ActivationFunctionType.Exp
    Ln = mybir.ActivationFunctionType.Ln

    with tile.TileContext(nc) as tc:
        import contextlib

        with contextlib.ExitStack() as ctx:
            consts = ctx.enter_context(tc.tile_pool(name="consts", bufs=1))
            wpool = ctx.enter_context(tc.tile_pool(name="wpool", bufs=1))
            xy = ctx.enter_context(tc.tile_pool(name="xy", bufs=6))
            qkv = ctx.enter_context(tc.tile_pool(name="qkv", bufs=1))
            epool = ctx.enter_context(tc.tile_pool(name="epool", bufs=2))
            onorm = ctx.enter_context(tc.tile_pool(name="onorm", bufs=1))
            npool = ctx.enter_context(tc.tile_pool(name="npool", bufs=1))
            stage = ctx.enter_context(tc.tile_pool(name="stage", bufs=3))

            # ---- phase 0: constants ------------------------------------
            ones = consts.tile([128, 1], BF16, tag="ones")
            nc.vector.memset(ones, 1.0)
            ones_row = consts.tile([1, 128], F32, tag="ones_row")
            nc.vector.memset(ones_row, 1.0)
            ones_full = consts.tile([128, 128], F32, tag="ones_full")
            nc.vector.memset(ones_full, 1.0)
            t_tau = consts.tile([1, 1], F32, tag="t_tau")
            nc.sync.dma_start(t_tau[:], tau_in[:])
            t_e = consts.tile([1, 1], F32, tag="t_e")
            nc.scalar.activation(t_e[:], t_tau[:], Exp)
            t_sp = consts.tile([1, 1], F32, tag="t_sp")
            nc.scalar.activation(t_sp[:], t_e[:], Ln, bias=1.0)
            t_sp2 = consts.tile([1, 1], F32, tag="t_sp2")
            nc.vector.tensor_scalar_add(t_sp2[:], t_sp[:], 1e-6)
            t_inv = consts.tile([1, 1], F32, tag="t_inv")
            nc.vector.reciprocal(t_inv[:], t_sp2[:])
            t_s1 = consts.tile([1, 1], F32, tag="t_s1")
            nc.vector.tensor_scalar_mul(t_s1[:], t_inv[:], float(D**-0.5))
            scale = consts.tile([128, 1], F32, tag="scale")
            with tc.tile_pool(name="ps_c", bufs=1, space="PSUM") as ps_c:
                sc_ps = ps_c.tile([128, 1], F32, tag="sc_ps")
                nc.tensor.matmul(sc_ps[:], ones_row[:], t_s1[:])
                nc.vector.tensor_copy(scale[:], sc_ps[:])

            # ---- weights ----------------------------------------------
            w_all = wpool.tile([128, CC, WQKV_W], BF16, tag="w_all")
            for c in range(CC):
                nc.sync.dma_start(
                    w_all[:, c, :], wqkvT[c * 128 : (c + 1) * 128, :]
                )
            wp_sb = wpool.tile([128, PAIRS * C], BF16, tag="wp_sb")
            nc.sync.dma_start(wp_sb[:], wp[:])

            def wq_sl(c, p):
                return w_all[:, c, p * 128 : (p + 1) * 128]

            def wk_sl(c, p):
                off = HPC * D
                return w_all[:, c, off + p * 128 : off + (p + 1) * 128]

            def wv_sl(c):
                off = 2 * HPC * D
                return w_all[:, c, off : off + HPC * D]

            # resident Q^T/K^T/V tiles
            qT = [qkv.tile([128, N], BF16, tag=f"qT{p}", name=f"qT{p}") for p in range(PAIRS)]
            kT = [qkv.tile([128, N], BF16, tag=f"kT{p}", name=f"kT{p}") for p in range(PAIRS)]
            v_sb = [qkv.tile([128, N], BF16, tag=f"v{p}", name=f"v{p}") for p in range(PAIRS)]
            o_nrm = [onorm.tile([128, N], BF16, tag=f"on{p}", name=f"on{p}") for p in range(PAIRS)]

            # ---- phase 1: prelude (V, K^T, Q^T) ------------------------
            with (
                tc.tile_pool(name="ps_pre", bufs=4, space="PSUM") as ps_pre,
                tc.tile_pool(name="ps_v", bufs=2, space="PSUM") as ps_v,
            ):
                yts = [xy.tile([128, N], BF16, tag="xy", name="xy") for _ in range(CC)]
                for c in range(CC):
                    nc.sync.dma_start(yts[c][:], yT[c * 128 : (c + 1) * 128, :])

                # V natural: [m, d] accumulated over c
                for mc in range(MC):
                    pv = ps_v.tile([128, HPC * D], F32, tag="pv")
                    for c in range(CC):
                        nc.tensor.matmul(
                            pv[:],
                            yts[c][:, mc * 128 : (mc + 1) * 128],
                            wv_sl(c),
                            start=(c == 0),
                            stop=(c == CC - 1),
                        )
                    for p in range(PAIRS):
                        nc.vector.tensor_copy(
                            v_sb[p][:, mc * 128 : (mc + 1) * 128],
                            pv[:, p * 128 : (p + 1) * 128],
                        )

                # K^T
                for p in range(PAIRS):
                    pk = [ps_pre.tile([128, 512], F32, tag="pre", name="pre") for _ in range(NB)]
                    for c in range(CC):
                        for nb in range(NB):
                            nc.tensor.matmul(
                                pk[nb][:],
                                wk_sl(c, p),
                                yts[c][:, nb * 512 : (nb + 1) * 512],
                                start=(c == 0),
                                stop=(c == CC - 1),
                            )
                    for nb in range(NB):
                        nc.vector.tensor_copy(
                            kT[p][:, nb * 512 : (nb + 1) * 512], pk[nb][:]
                        )

                # Q^T (xT replaces yT in the xy pool)
                xts = [xy.tile([128, N], BF16, tag="xy", name="xy") for _ in range(CC)]
                for c in range(CC):
                    nc.sync.dma_start(xts[c][:], xT[c * 128 : (c + 1) * 128, :])
                for p in range(PAIRS):
                    pq = [ps_pre.tile([128, 512], F32, tag="pre", name="pre") for _ in range(NB)]
                    for c in range(CC):
                        for nb in range(NB):
                            nc.tensor.matmul(
                                pq[nb][:],
                                wq_sl(c, p),
                                xts[c][:, nb * 512 : (nb + 1) * 512],
                                start=(c == 0),
                                stop=(c == CC - 1),
                            )
                    for nb in range(NB):
                        nc.vector.tensor_copy(
                            qT[p][:, nb * 512 : (nb + 1) * 512], pq[nb][:]
                        )

            if DEBUG_DUMP:
                nc.sync.dma_start(dbg_qT[:], qT[0][:])
                nc.sync.dma_start(dbg_kT[:], kT[0][:])
                nc.sync.dma_start(dbg_v[:], v_sb[0][:])
                nc.sync.dma_start(dbg_scale[:], scale[:])

            # ---- phase 2: attention pairs ------------------------------
            with (
                tc.tile_pool(name="ps_acc", bufs=1, space="PSUM") as ps_acc,
                tc.tile_pool(name="ps_s", bufs=1, space="PSUM") as ps_s,
                tc.tile_pool(name="ps_bc", bufs=1, space="PSUM") as ps_bc,
            ):
                for p in range(PAIRS):
                    for hf in range(2):  # q-halves of 1024
                        q0 = hf * 1024
                        o_qb = [
                            ps_acc.tile([128, 512], F32, tag=f"oq{qb}", name=f"oq{qb}")
                            for qb in range(2)
                        ]
                        rs = ps_acc.tile([128, 512], F32, tag="rs")
                        for mc in range(MC):
                            m0 = mc * 128
                            s_a = ps_s.tile([128, 1024], F32, tag="sa")
                            s_b = ps_s.tile([128, 1024], F32, tag="sb")
                            for qb in range(2):
                                nc.tensor.matmul(
                                    s_a[:, qb * 512 : (qb + 1) * 512],
                                    kT[p][0:64, m0 : m0 + 128],
                                    qT[p][0:64, q0 + qb * 512 : q0 + (qb + 1) * 512],
                                    tile_position=(0, 0),
                                )
                                nc.tensor.matmul(
                                    s_b[:, qb * 512 : (qb + 1) * 512],
                                    kT[p][64:128, m0 : m0 + 128],
                                    qT[p][64:128, q0 + qb * 512 : q0 + (qb + 1) * 512],
                                    tile_position=(64, 0),
                                )
                            e_a = epool.tile([128, 1024], BF16, tag="ea")
                            e_b = epool.tile([128, 1024], BF16, tag="eb")
                            nc.scalar.activation(
                                e_a[:], s_a[:], Exp, scale=scale[:]
                            )
                            nc.scalar.activation(
                                e_b[:], s_b[:], Exp, scale=scale[:]
                            )
                            if DEBUG_DUMP and p == 0 and hf == 0 and mc == 0:
                                nc.sync.dma_start(dbg_e[:], e_a[:])
                            st = dict(
                                start=(mc == 0),
                                stop=(mc == MC - 1),
                                skip_group_check=True,
                            )
                            for qb in range(2):
                                nc.tensor.matmul(
                                    o_qb[qb][0:64, :],
                                    v_sb[p][:, m0 : m0 + 64],
                                    e_a[:, qb * 512 : (qb + 1) * 512],
                                    tile_position=(0, 0),
                                    **st,
                                )
                            for qb in range(2):
                                nc.tensor.matmul(
                                    o_qb[qb][64:128, :],
                                    v_sb[p][:, m0 + 64 : m0 + 128],
                                    e_b[:, qb * 512 : (qb + 1) * 512],
                                    tile_position=(0, 64),
                                    **st,
                                )
                            # rowsums: rows 0/32 = qb0 (A/B), 64/96 = qb1
                            for qb in range(2):
                                for hd, e_t in ((0, e_a), (1, e_b)):
                                    row = qb * 64 + hd * 32
                                    nc.tensor.matmul(
                                        rs[row : row + 1, :],
                                        ones[:, 0:1],
                                        e_t[:, qb * 512 : (qb + 1) * 512],
                                        tile_position=(0, row),
                                        **st,
                                    )
                        if DEBUG_DUMP and p == 0 and hf == 0:
                            rs_st = stage.tile([128, 512], F32, tag="rs_st")
                            nc.vector.tensor_copy(rs_st[:], rs[:])
                            nc.sync.dma_start(dbg_rs[:], rs_st[:])
                            ou_st = stage.tile([128, 512], F32, tag="ou_st")
                            nc.vector.tensor_copy(ou_st[:], o_qb[0][:])
                            nc.sync.dma_start(dbg_ou[:], ou_st[:])
                        # normalize both q-blocks of this half
                        rr_all = npool.tile(
                            [128, 512], F32, tag="rr_all", name="rr_all"
                        )
                        nc.vector.reciprocal(
                            rr_all[0:97, :], rs[0:97, :]
                        )
                        for qb in range(2):
                            sl = slice(q0 + qb * 512, q0 + (qb + 1) * 512)
                            for hd in range(2):
                                row = qb * 64 + hd * 32
                                bc_ps = ps_bc.tile(
                                    [128, 512], F32, tag="bc_ps", name="bc_ps"
                                )
                                nc.tensor.matmul(
                                    bc_ps[:],
                                    ones_full[row : row + 1, 0:128],
                                    rr_all[row : row + 1, :],
                                    tile_position=(96, 0) if row == 96 else None,
                                )
                                bc = npool.tile(
                                    [128, 512], F32, tag=f"bc{hd}", name=f"bc{hd}"
                                )
                                nc.vector.tensor_copy(bc[:], bc_ps[:])
                                if (
                                    DEBUG_DUMP
                                    and p == 0
                                    and hf == 0
                                    and qb == 0
                                    and hd == 0
                                ):
                                    nc.sync.dma_start(dbg_rr[:], rr_all[0:1, :])
                                    nc.sync.dma_start(dbg_bc[:], bc[:])
                                if (
                                    DEBUG_DUMP
                                    and p == 0
                                    and hf == 0
                                    and qb == 1
                                    and hd == 0
                                ):
                                    nc.sync.dma_start(dbg_bc2[:], bc[:])
                                hs = slice(hd * 64, (hd + 1) * 64)
                                nc.vector.tensor_mul(
                                    o_nrm[p][hs, sl], o_qb[qb][hs, :], bc[hs, :]
                                )

            if DEBUG_DUMP:
                nc.sync.dma_start(dbg_on[:], o_nrm[0][:])

            # ---- phase 3: output projection ----------------------------
            with tc.tile_pool(name="ps_out", bufs=4, space="PSUM") as ps_out:
                for ic in range(CC):
                    for nb in range(NB):
                        po = ps_out.tile([128, 512], F32, tag="po")
                        for p in range(PAIRS):
                            nc.tensor.matmul(
                                po[:],
                                wp_sb[:, p * C + ic * 128 : p * C + (ic + 1) * 128],
                                o_nrm[p][:, nb * 512 : (nb + 1) * 512],
                                start=(p == 0),
                                stop=(p == PAIRS - 1),
                            )
                        so = stage.tile([128, 512], F32, tag="so")
                        nc.scalar.copy(so[:], po[:])
                        nc.sync.dma_start(
                            outT[
                                ic * 128 : (ic + 1) * 128,
                                nb * 512 : (nb + 1) * 512,
                            ],
                            so[:],
                        )
    nc.compile()
    return nc


_NC = None


def _get_nc():
    global _NC
    if _NC is None:
        _NC = _build()
    return _NC


def kernel(x, y, Wq, Wkv, tau_param, Wproj, bproj):
    x = np.asarray(x, np.float32)
    y = np.asarray(y, np.float32)
    Wq = np.asarray(Wq, np.float32)
    Wkv = np.asarray(Wkv, np.float32)
    Wproj = np.asarray(Wproj, np.float32)
    bproj = np.asarray(bproj, np.float32)
    tau_np = np.asarray(tau_param, np.float32).reshape(1, 1)

    import ml_dtypes

    in_maps = []
    for c in range(8):
        b = c // 2
        h0 = (c % 2) * HPC
        rows = slice(h0 * D, h0 * D + HPC * D)
        wq_s = Wq[rows, :].T  # [C, 384]
        wk_s = Wkv[rows, :].T
        wv_s = Wkv[C + h0 * D : C + h0 * D + HPC * D, :].T
        wqkvT = np.ascontiguousarray(
            np.concatenate([wq_s, wk_s, wv_s], axis=1)
        ).astype(ml_dtypes.bfloat16)
        wpT = Wproj[:, h0 * D : h0 * D + HPC * D].T  # [384, C]
        wp_packed = np.empty((128, PAIRS * C), ml_dtypes.bfloat16)
        for p in range(PAIRS):
            wp_packed[:, p * C : (p + 1) * C] = wpT[
                p * 128 : (p + 1) * 128, :
            ].astype(ml_dtypes.bfloat16)
        in_maps.append(
            {
                "xT": np.ascontiguousarray(x[b].T).astype(ml_dtypes.bfloat16),
                "yT": np.ascontiguousarray(y[b].T).astype(ml_dtypes.bfloat16),
                "wqkvT": wqkvT,
                "wp": wp_packed,
                "tau_in": tau_np,
            }
        )

    nc = _get_nc()
    trace = bool(int(os.environ.get("KERNEL_PROFILE", "0")))
    if trace:
        _install_ntff_shim()
    res = run_bass_kernel_spmd(nc, in_maps, list(range(8)), trace=trace)
    kernel.last_results = res.results
    if trace and res.exec_time_ns is not None:
        print(f"HW exec time: {res.exec_time_ns} ns")
        kernel.last_exec_time_ns = res.exec_time_ns
        kernel.last_trace = res.instructions_and_trace
        kernel.last_profile_json = res.profile_json

    out = np.empty((B, N, C), np.float32)
    for b in range(B):
        acc = res.results[2 * b]["outT"].T + res.results[2 * b + 1]["outT"].T
        out[b] = acc + bproj[None, :]
    return out


def _install_ntff_shim():
    import sys
    import types

    try:
        from antenv import axon_hooks  # noqa: F401

        return
    except ImportError:
        pass
    from trn_agent_boot.trn_boot import _ntff_profile_via_ctypes

    hook = _ntff_profile_via_ctypes("/opt/axon/libaxon_pjrt.so")
    mod = types.ModuleType("antenv.axon_hooks")
    mod.get_axon_ntff_profile_hook = lambda: hook
    mod.set_axon_ntff_profile_hook = lambda h: None
    sys.modules["antenv.axon_hooks"] = mod
    import concourse.bass_utils as bu

    bu.upload_artifacts = lambda tmpdir: "local://" + str(tmpdir)
